# revision 16
# baseline (speedup 1.0000x reference)
"""Trainium2 Bass kernel for ContextQueryAttention (BiDAF-style trilinear attention).

Computes, per batch n:
    sim[c,q] = <ctx[c], wc> + <xq[q], wc> + <ctx[c] * wcq, xq[q]>
    c2q  = softmax_q(sim) @ xq                      # [C, F]
    q2c  = softmax_c(max_q sim) @ ctx               # [F]
    out  = concat([ctx, c2q, ctx*c2q, ctx*q2c], -1) # [C, 4F]

Sharding: data-parallel over batch N=64 across 8 NeuronCores (8 batches/core).

fp16 end-to-end (inputs cast on host, fp16 matmuls/stores, fp32 PSUM + softmax
stats). The device computes the three derived F-blocks [c2q, ctx*c2q,
ctx*q2c]; the passthrough block out[:, :, 0:F] = x_context is filled on the
host during unshard (it is an exact copy of the input -- no compute).

Per-core structure (per batch, all DMA on the sync ring):
  - ctx loads prefetched one batch ahead; xqT loaded pre-transposed via the
    XBAR dma transpose (chunk layout f = fc*128 + p matches the weight
    chunks), so xqw_aug = [wcq*xqT | wc] is two DVE ops
  - ctxT built via 32 PE transposes, evicted per f-chunk as [128, 1024]
  - sim psum [128c, 129] per c-tile: 4 K-chunk matmuls with the augmented
    moving operand so column 128 accumulates s_ctx for free, plus a rank-1
    (ones x s_qry) matmul
  - softmax over q on the free axis: DVE reduce_max(negate) -> ACT exp with
    per-partition bias and accumulated row-sum
  - pass 2 software-pipelined: E^T PE-transposed in pairs one pair ahead of
    the c2q matmuls; normalization fused into the psum->sbuf copy; the
    [c2q | ctx*c2q] halves stored per tile as soon as ready
  - q2c: global softmax over the 1024 context logits z = s_ctx + rowmax
    (gpsimd partition_all_reduce), rank-1 matmuls against natural ctx,
    ones-matmul partition-broadcast; term4 split DVE/gpsimd, stored per tile
  - last batch: q2c chain emitted before pass 2 and term4+store trail the
    c2q stages by two tiles, so the tail drains early
  - PSUM pools: sim / ctxT / big(c2q+q2c chain) / small(prep+ET) -- prep of
    batch b+1 does not wait on the q2c chain of batch b
"""

import os

os.environ.setdefault("JAX_PLATFORMS", "axon")

import numpy as np

import concourse.bass as bass
import concourse.mybir as mybir
import concourse.tile as tile
from concourse import bacc, bass_isa, bass_utils
from concourse.masks import make_identity

f32 = mybir.dt.float32
f16 = mybir.dt.float16
AX = mybir.AxisListType.X
EXP = mybir.ActivationFunctionType.Exp
COPY = mybir.ActivationFunctionType.Copy

N_CORES = 8
B = 8          # batches per core
C = 1024       # context length
Q = 128        # query length
F = 512        # feature dim
CT = C // 128  # c-tiles per batch
FC = F // 128  # f-chunks


def build_nc():
    nc = bacc.Bacc("TRN2", target_bir_lowering=False, debug=False)
    xc = nc.dram_tensor("x_context", [B, C, F], f16, kind="ExternalInput").ap()
    xq_d = nc.dram_tensor("x_query", [B, Q, F], f16, kind="ExternalInput").ap()
    wc_d = nc.dram_tensor("w_context", [F], f32, kind="ExternalInput").ap()
    wcq_d = nc.dram_tensor("w_cq", [F], f32, kind="ExternalInput").ap()
    out = nc.dram_tensor("out", [B, C, 3 * F], f16, kind="ExternalOutput").ap()

    from contextlib import ExitStack

    with tile.TileContext(nc) as tc, ExitStack() as es:
        def pool(name, bufs, space="SBUF"):
            return es.enter_context(tc.tile_pool(name=name, bufs=bufs, space=space))

        const = pool("const", 1)
        ctx_p = pool("ctx_p", 3)
        ctxT_p = pool("ctxT_p", 2)
        xq_p = pool("xq_p", 3)
        xqw_p = pool("xqw_p", 2)
        tmp_p = pool("tmp_p", 2)
        e_p = pool("e_p", CT + 2)
        et_p = pool("et_p", 3)
        asm_p = pool("asm_p", CT + 1)
        vec_p = pool("vec_p", CT + 2)
        sml_p = pool("sml_p", 2)
        ps_sim_p = pool("ps_sim", 2, "PSUM")
        ps_ctxT_p = pool("ps_ctxT", 2, "PSUM")
        ps_big_p = pool("ps_big", 2, "PSUM")
        ps_sml_p = pool("ps_sml", 2, "PSUM")

        dma_load = nc.sync.dma_start
        dma_store = nc.sync.dma_start

        ident = const.tile([128, 128], f16)
        make_identity(nc, ident)
        ident32 = const.tile([128, 128], f32)
        make_identity(nc, ident32)
        ones_row = const.tile([1, 128], f16)
        nc.vector.memset(ones_row, 1.0)
        ones_row32 = const.tile([1, 128], f32)
        nc.vector.memset(ones_row32, 1.0)
        ones_col = const.tile([128, 1], f32)
        nc.vector.memset(ones_col, 1.0)
        wc_sb = const.tile([128, FC], f32)
        dma_load(wc_sb, wc_d.rearrange("(a p) -> p a", p=128))
        wcq_sb = const.tile([128, FC], f32)
        dma_load(wcq_sb, wcq_d.rearrange("(a p) -> p a", p=128))
        wc_row = const.tile([1, F], f32)
        dma_load(wc_row, wc_d[None, :])
        # wc broadcast along partitions (for s_qry): ones[1,128]^T @ wc[1,512]
        ps_wcb = ps_big_p.tile([128, F], f32, tag="big")
        nc.tensor.matmul(ps_wcb, lhsT=ones_row32, rhs=wc_row, start=True, stop=True)
        wc_bc = const.tile([128, F], f16)
        nc.vector.tensor_copy(wc_bc, ps_wcb)

        def load_batch(b):
            ctx = ctx_p.tile([128, CT, F], f16, name="ctx")
            dma_load(ctx, xc[b].rearrange("(t p) f -> p t f", p=128))
            xq = xq_p.tile([128, F], f16, name="xq")
            dma_load(xq, xq_d[b])
            return ctx, xq

        pending = [load_batch(0), load_batch(1)]
        for b in range(B):
            # ---- loads (prefetched two batches ahead: the sync ring carries
            # ~9us of stores ahead of each load, so depth-1 prefetch arrives
            # late and stalls the ctxT transposes) ----
            ctx, xq = pending.pop(0)
            if b + 2 < B:
                pending.append(load_batch(b + 2))

            # ---- xqT, scaled by w_cq, augmented with wc column ----
            # xqw_aug[:, fc] = [wcq*xqT chunk | wc chunk]   ([128, 129])
            xqw_aug = xqw_p.tile([128, FC, Q + 1], f16)
            for fc in range(FC):
                ps_xqT = ps_sml_p.tile([128, 128], f16, tag="sml")
                nc.tensor.transpose(ps_xqT, xq[:, fc * 128 : (fc + 1) * 128], ident)
                nc.scalar.activation(
                    xqw_aug[:, fc, 0:Q], ps_xqT, COPY,
                    scale=wcq_sb[:, fc : fc + 1],
                )
                nc.vector.tensor_copy(
                    xqw_aug[:, fc, Q : Q + 1], wc_sb[:, fc : fc + 1]
                )

            # ---- s_qry row [1, 128] (fused mul+reduce, then PE transpose) ----
            scr = tmp_p.tile([128, F], f16, name="scr", tag="scr")
            sq_col = vec_p.tile([128, 1], f32, tag="sqcol")
            nc.vector.tensor_mul(scr, xq, wc_bc)
            nc.vector.reduce_sum(sq_col, scr, axis=AX)
            ps_sqT = ps_sml_p.tile([1, 128], f32, tag="sml")
            nc.tensor.transpose(ps_sqT, sq_col, ident32)
            sq_row = sml_p.tile([1, 128], f16, name="sq_row", tag="sq_row")
            nc.scalar.copy(sq_row, ps_sqT)

            # ---- ctxT [f, c]: per f-chunk, 8 transposes + one [128,1024] evict ----
            ctxT = ctxT_p.tile([128, FC, C], f16)
            for fc in range(FC):
                ps_ct = ps_ctxT_p.tile([128, C], f16)
                for t in range(CT):
                    nc.tensor.transpose(
                        ps_ct[:, t * 128 : (t + 1) * 128],
                        ctx[:, t, fc * 128 : (fc + 1) * 128],
                        ident,
                    )
                cp = nc.vector.tensor_copy if fc % 2 == 0 else nc.scalar.copy
                cp(ctxT[:, fc, :], ps_ct)

            # ---- pass 1: sim + softmax stats per c-tile ----
            z = sml_p.tile([128, CT], f32, name="z", tag="z")
            Es = []
            rcps = []
            for t in range(CT):
                ps_sim = ps_sim_p.tile([128, Q + 1], f32)
                for fc in range(FC):
                    nc.tensor.matmul(
                        ps_sim,
                        lhsT=ctxT[:, fc, t * 128 : t * 128 + 128],
                        rhs=xqw_aug[:, fc],
                        start=(fc == 0),
                        stop=False,
                    )
                nc.tensor.matmul(
                    ps_sim[:, 0:Q], lhsT=ones_row, rhs=sq_row, start=False, stop=True
                )
                nmax = vec_p.tile([128, 1], f32, tag="nmax")
                nc.vector.reduce_max(nmax, ps_sim[:, 0:Q], axis=AX, negate=True)
                E = e_p.tile([128, Q], f16)
                rsum = vec_p.tile([128, 1], f32, tag="rsum")
                nc.scalar.activation(E, ps_sim[:, 0:Q], EXP, bias=nmax, accum_out=rsum)
                rcp = vec_p.tile([128, 1], f32, tag="rcp")
                nc.vector.reciprocal(rcp, rsum)
                # z[:, t] = s_ctx + rowmax = psum[:,128] - (-max)
                nc.vector.tensor_sub(z[:, t : t + 1], ps_sim[:, Q : Q + 1], nmax)
                Es.append(E)
                rcps.append(rcp)

            # ---- q2c softmax prep ----
            zmax = vec_p.tile([128, 1], f32, tag="zmax")
            nc.vector.reduce_max(zmax, z, axis=AX)
            gmax = vec_p.tile([128, 1], f32, tag="gmax")
            nc.gpsimd.partition_all_reduce(
                gmax, zmax, channels=128, reduce_op=bass_isa.ReduceOp.max
            )
            negb = vec_p.tile([128, 1], f32, tag="negb")
            nc.vector.tensor_scalar_mul(negb, gmax, -1.0)
            expz = sml_p.tile([128, CT], f16, name="expz", tag="expz")
            ers = vec_p.tile([128, 1], f32, tag="ers")
            nc.scalar.activation(expz, z, EXP, bias=negb, accum_out=ers)

            def emit_q2c():
                # q2c weighted sum (PE rank-1 matmuls) + ones-matmul broadcast
                ps_S = ps_sml_p.tile([1, 1], f32, tag="sml")
                nc.tensor.matmul(ps_S, lhsT=ers, rhs=ones_col, start=True, stop=True)
                rS = sml_p.tile([1, 1], f32, name="rS", tag="rS")
                nc.vector.reciprocal(rS, ps_S)
                ps_q2c = ps_big_p.tile([1, F], f32, tag="big")
                for t in range(CT):
                    nc.tensor.matmul(
                        ps_q2c,
                        lhsT=expz[:, t : t + 1],
                        rhs=ctx[:, t],
                        start=(t == 0),
                        stop=(t == CT - 1),
                    )
                xq2c = sml_p.tile([1, F], f16, name="xq2c", tag="xq2c")
                nc.scalar.activation(xq2c, ps_q2c, COPY, scale=rS)
                ps_bc = ps_big_p.tile([128, F], f32, tag="big")
                nc.tensor.matmul(ps_bc, lhsT=ones_row, rhs=xq2c, start=True, stop=True)
                xq2cb = tmp_p.tile([128, F], f16, name="xq2cb", tag="xq2cb")
                nc.vector.tensor_copy(xq2cb, ps_bc)
                return xq2cb

            # ---- pass 2 (software-pipelined): E^T pairs one ahead of c2q ----
            asms = []

            def stage_et_pair(p):
                ps_et = ps_sml_p.tile([128, 2, Q], f16, tag="sml")
                nc.tensor.transpose(ps_et[:, 0, :], Es[2 * p], ident)
                nc.tensor.transpose(ps_et[:, 1, :], Es[2 * p + 1], ident)
                ET2 = et_p.tile([128, 2, Q], f16)
                cp = nc.scalar.copy if p % 2 == 0 else nc.vector.tensor_copy
                cp(ET2, ps_et)
                return ET2

            def stage_c2q(t, ET):
                ps_c2q = ps_big_p.tile([128, F], f32, tag="big")
                nc.tensor.matmul(ps_c2q, lhsT=ET, rhs=xq, start=True, stop=True)
                asm = asm_p.tile([128, 3 * F], f16)
                # normalized c2q, fused into the psum->sbuf move
                if t % 2 == 0:
                    nc.scalar.activation(asm[:, 0:F], ps_c2q, COPY, scale=rcps[t])
                else:
                    nc.vector.tensor_scalar_mul(asm[:, 0:F], ps_c2q, rcps[t])
                nc.vector.tensor_mul(asm[:, F : 2 * F], ctx[:, t], asm[:, 0:F])
                # early half-store: keeps the store ring streaming
                dma_store(
                    out[b, t * 128 : (t + 1) * 128, 0 : 2 * F], asm[:, 0 : 2 * F]
                )
                asms.append(asm)

            def stage_fin(t, xq2cb):
                asm = asms[t]
                eng = nc.vector if t % 8 < 3 else nc.gpsimd
                eng.tensor_mul(asm[:, 2 * F : 3 * F], ctx[:, t], xq2cb)
                dma_store(
                    out[b, t * 128 : (t + 1) * 128, 2 * F : 3 * F],
                    asm[:, 2 * F : 3 * F],
                )

            if b == B - 1:
                # last batch: q2c chain first, term4+store trails c2q by 2 tiles
                xq2cb = emit_q2c()
                prev = stage_et_pair(0)
                for p in range(CT // 2):
                    nxt_et = stage_et_pair(p + 1) if p + 1 < CT // 2 else None
                    stage_c2q(2 * p, prev[:, 0, :])
                    stage_c2q(2 * p + 1, prev[:, 1, :])
                    prev = nxt_et
                    for t in (2 * p - 2, 2 * p - 1):
                        if t >= 0:
                            stage_fin(t, xq2cb)
                for t in (CT - 2, CT - 1):
                    stage_fin(t, xq2cb)
            else:
                prev = stage_et_pair(0)
                for p in range(CT // 2):
                    nxt_et = stage_et_pair(p + 1) if p + 1 < CT // 2 else None
                    stage_c2q(2 * p, prev[:, 0, :])
                    stage_c2q(2 * p + 1, prev[:, 1, :])
                    prev = nxt_et
                xq2cb = emit_q2c()
                for t in range(CT):
                    stage_fin(t, xq2cb)

    nc.compile()
    return nc


_NC = None


def make_in_maps(inputs):
    xc = np.ascontiguousarray(np.asarray(inputs["x_context"]), dtype=np.float16)
    xq = np.ascontiguousarray(np.asarray(inputs["x_query"]), dtype=np.float16)
    wc = np.ascontiguousarray(np.asarray(inputs["w_context"], dtype=np.float32))
    wcq = np.ascontiguousarray(np.asarray(inputs["w_cq"], dtype=np.float32))
    return [
        {
            "x_context": xc[i * B : (i + 1) * B],
            "x_query": xq[i * B : (i + 1) * B],
            "w_context": wc,
            "w_cq": wcq,
        }
        for i in range(N_CORES)
    ]


def kernel(**inputs):
    global _NC
    if _NC is None:
        _NC = build_nc()
    in_maps = make_in_maps(inputs)
    res = bass_utils.run_bass_kernel_spmd(_NC, in_maps, core_ids=list(range(N_CORES)))
    xc_full = np.asarray(inputs["x_context"], dtype=np.float32)
    N = xc_full.shape[0]
    full = np.empty((N, C, 4 * F), dtype=np.float32)
    # out[:, :, 0:F] is the passthrough block: exactly x_context
    full[:, :, 0:F] = xc_full
    dev = np.concatenate([res.results[i]["out"] for i in range(N_CORES)], axis=0)
    full[:, :, F:] = dev
    return full
